# revision 1
# baseline (speedup 1.0000x reference)
"""Trainium2 Bass kernel for NeuralGraphHidden (GNN message passing).

Full-input contract: kernel(**inputs) takes the complete unsharded arrays,
shards batch dim 0 across 8 NeuronCores (data parallel), runs one SPMD Bass
program, and reassembles the full output.

Math (per molecule, A=128 atoms, D=5 degree slots):
  deg[a]      = #(edges[a,:] != -1)
  neigh[a]    = atoms[a] + sum_d atoms[edges[a,d]]        (-1 -> no contribution)
  sbond[a]    = sum_d bonds[a,d]
  feat[a]     = [neigh[a], sbond[a], 1.0]                 (bias folded as feature)
  Z_d[a]      = feat[a] @ Waug[d]                          (Waug = [W;b])
  out[a]      = relu(Z_{deg[a]}[a])  if deg[a] < 5 else 0

On-chip formulation:
  ET'[a',a] = I + sum_d onehot(edges[a,d])[a']   built via DVE is_equal vs iota
  neighT    = atoms_chunk.T @ ET'                (TensorE, contracts atoms axis)
  deg+1     = ones.T-col-sums of ET'             (TensorE)
  bondsT    = 5 accumulating transposes          (TensorE, rhs=I)
  Z         = featT.T @ Waug                     (TensorE, 3 K-chunks x 5 degrees)
  select    = sum_d diag(deg==d) @ Z_d           (TensorE, PSUM-accumulated;
                                                  exact: relu and select commute)
  out       = relu(select)                       (ScalarE)
"""

import sys

sys.path.insert(0, "/opt/trn_rl_repo")

import numpy as np

B, A, D = 256, 128, 5
FA, FB, C = 256, 64, 256
F = FA + FB        # 320
FAUG = F + 1       # 321 (bias row)
NCORES = 8
BL = B // NCORES   # 32 molecules per core

_CACHE = {}


def _build_program():
    from contextlib import ExitStack

    import concourse.bass as bass
    import concourse.tile as tile
    from concourse import bacc, mybir

    f32 = mybir.dt.float32
    i32 = mybir.dt.int32
    AF = mybir.ActivationFunctionType
    OP = mybir.AluOpType
    # float32r operands: single-pass (TF32-like) PE matmul at 2x fp32 rate;
    # every matmul operand below is produced/declared as f32r to satisfy the
    # BIR verifier's rounding rule. One-hot/mask/count values are small
    # integers, exactly representable at reduced mantissa, so the gather and
    # degree-select logic stays exact.
    f32r = mybir.dt.float32r
    bf16 = mybir.dt.bfloat16

    nc = bacc.Bacc("TRN2", target_bir_lowering=False, debug=False,
                   num_devices=NCORES)

    atoms_d = nc.dram_tensor("atoms", [BL, A, FA], f32r, kind="ExternalInput")
    bonds_d = nc.dram_tensor("bonds", [BL, A, D * FB], f32,
                             kind="ExternalInput")
    # edge indices as bf16 (exact for -1..127), host-replicated down
    # partitions in (d, a)-major order for the 2x-mode add tree
    edges_d = nc.dram_tensor("edges", [BL, A, A * D], bf16,
                             kind="ExternalInput")
    waug_d = nc.dram_tensor("waug", [D, FAUG, C], f32r, kind="ExternalInput")
    ident_d = nc.dram_tensor("ident", [A, A], f32, kind="ExternalInput")
    identr_d = nc.dram_tensor("identr", [A, A], f32r, kind="ExternalInput")
    identb_d = nc.dram_tensor("identb", [A, A], bf16, kind="ExternalInput")
    iota_d = nc.dram_tensor("iota", [A, 1], f32, kind="ExternalInput")
    edeg_d = nc.dram_tensor("edeg", [BL, A, D], f32, kind="ExternalInput")
    onesr_d = nc.dram_tensor("onesr", [1, A], f32, kind="ExternalInput")
    out_d = nc.dram_tensor("out", [BL, A, C], f32, kind="ExternalOutput")

    atoms_ap = atoms_d.ap()
    bonds_ap = bonds_d.ap()
    edges_ap = edges_d.ap()
    out_ap = out_d.ap()

    with tile.TileContext(nc) as tc, ExitStack() as ctx:
        consts = ctx.enter_context(tc.tile_pool(name="consts", bufs=1))
        pin = ctx.enter_context(tc.tile_pool(name="pin", bufs=3))
        pbc = ctx.enter_context(tc.tile_pool(name="pbc", bufs=2))
        pet = ctx.enter_context(tc.tile_pool(name="pet", bufs=2))
        pfeat = ctx.enter_context(tc.tile_pool(name="pfeat", bufs=2))
        pmd = ctx.enter_context(tc.tile_pool(name="pmd", bufs=2))
        pz = ctx.enter_context(tc.tile_pool(name="pz", bufs=2))
        pout = ctx.enter_context(tc.tile_pool(name="pout", bufs=3))
        ps_f = ctx.enter_context(
            tc.tile_pool(name="ps_f", bufs=2, space="PSUM"))
        ps_c2 = ctx.enter_context(
            tc.tile_pool(name="ps_c2", bufs=1, space="PSUM"))
        ps_z = ctx.enter_context(
            tc.tile_pool(name="ps_z", bufs=1, space="PSUM"))
        ps_s = ctx.enter_context(
            tc.tile_pool(name="ps_s", bufs=1, space="PSUM"))

        # ---- one-time setup -------------------------------------------------
        # Constants + weights issue from the Scalar/GpSimd engine queues so
        # the Sync queue serves only per-group input loads (startup latency).
        G = 4
        ident = consts.tile([A, A], f32)
        nc.scalar.dma_start(out=ident[:], in_=ident_d.ap()[:])
        identr = consts.tile([A, A], f32r)
        nc.scalar.dma_start(out=identr[:], in_=identr_d.ap()[:])
        iota_col = consts.tile([A, 1], f32)
        nc.gpsimd.dma_start(out=iota_col[:], in_=iota_d.ap()[:])
        ones_row = consts.tile([1, A], f32)
        nc.scalar.dma_start(out=ones_row[:], in_=onesr_d.ap()[:])
        identb4 = consts.tile([A, G * A], bf16)
        for j in range(G):
            nc.gpsimd.dma_start(out=identb4[:, j * A:(j + 1) * A],
                                in_=identb_d.ap()[:])

        # Weights resident in SBUF: chunk k holds rows [k*128, ...) of Waug
        # for all 5 degrees side by side: w_k[:, d*256:(d+1)*256].
        # Issued from the Scalar engine queue so they don't delay the first
        # group's input loads on the Sync queue.
        w0 = consts.tile([128, D * C], f32r)
        w1 = consts.tile([128, D * C], f32r)
        w2 = consts.tile([FAUG - 256, D * C], f32r)  # 65 rows: 64 bond + bias
        for d in range(D):
            nc.scalar.dma_start(out=w0[:, d * C:(d + 1) * C],
                                in_=waug_d.ap()[d, 0:128, :])
            nc.scalar.dma_start(out=w1[:, d * C:(d + 1) * C],
                                in_=waug_d.ap()[d, 128:256, :])
            nc.scalar.dma_start(out=w2[:, d * C:(d + 1) * C],
                                in_=waug_d.ap()[d, 256:FAUG, :])

        # ---- molecules, in groups of 4 ----------------------------------
        # One DMA per group tensor (cuts SP issue cost 4x), batched DVE
        # compare/add-tree across the group, per-molecule matmul stages.
        for bg in range(BL // G):
            mols = range(bg * G, (bg + 1) * G)
            atoms4 = pin.tile([A, G * FA], f32r)
            nc.sync.dma_start(
                out=atoms4.rearrange("p (g f) -> p g f", g=G),
                in_=atoms_ap[bg * G:(bg + 1) * G].rearrange(
                    "g p f -> p g f"))
            bonds4 = pin.tile([A, G * D * FB], f32)
            nc.sync.dma_start(
                out=bonds4.rearrange("p (g f) -> p g f", g=G),
                in_=bonds_ap[bg * G:(bg + 1) * G].rearrange(
                    "g p f -> p g f"))
            bc_e4 = pbc.tile([A, G * A * D], bf16)
            nc.gpsimd.dma_start(
                out=bc_e4.rearrange("p (g f) -> p g f", g=G),
                in_=edges_ap[bg * G:(bg + 1) * G].rearrange(
                    "g p f -> p g f"))
            edeg4 = pfeat.tile([A, G * D], f32)
            nc.sync.dma_start(
                out=edeg4.rearrange("p (g f) -> p g f", g=G),
                in_=edeg_d.ap()[bg * G:(bg + 1) * G].rearrange(
                    "g p f -> p g f"))
            # deg+1 per molecule from the raw edge slots (tiny DVE chain)
            ne4 = pfeat.tile([A, G * D], f32)
            nc.vector.tensor_scalar(ne4[:], edeg4[:], -1.0, None,
                                    OP.not_equal)
            degp1_4 = pfeat.tile([A, G], f32)
            nc.vector.tensor_reduce(
                degp1_4[:], ne4.rearrange("p (g d) -> p g d", g=G),
                axis=mybir.AxisListType.X, op=OP.add)
            nc.vector.tensor_scalar(degp1_4[:], degp1_4[:], 1.0, None,
                                    OP.add)

            # one-hot compare + degree-slot sum (bf16 2x-mode add tree;
            # counts <= 6 are bf16-exact) + self term, batched per group
            cmp5 = pbc.tile([A, G * A * D], bf16)
            nc.vector.tensor_scalar(cmp5[:], bc_e4[:], iota_col[:], None,
                                    OP.is_equal)
            cg = cmp5.rearrange("p (g d a) -> p g d a", g=G, d=D)
            t01 = pet.tile([A, G * A], bf16)
            nc.vector.tensor_add(t01[:], cg[:, :, 0, :], cg[:, :, 1, :])
            t23 = pet.tile([A, G * A], bf16)
            nc.vector.tensor_add(t23[:], cg[:, :, 2, :], cg[:, :, 3, :])
            t4i = pet.tile([A, G * A], bf16)
            nc.vector.tensor_add(t4i[:], cg[:, :, 4, :], identb4[:])
            t0123 = pet.tile([A, G * A], bf16)
            nc.vector.tensor_add(t0123[:], t01[:], t23[:])
            etp4 = pet.tile([A, G * A], f32r)
            with nc.allow_low_precision(reason="exact small-int counts"):
                nc.vector.tensor_add(etp4[:], t0123[:], t4i[:])

            out4 = pout.tile([A, G * C], f32)
            for j, bm in enumerate(mols):
                etp = etp4[:, j * A:(j + 1) * A]
                atoms_sb = atoms4[:, j * FA:(j + 1) * FA]
                bonds_sb = bonds4[:, j * D * FB:(j + 1) * D * FB]

                degp1 = degp1_4[:, j:j + 1]

                # Neighbor+self sums (transposed) in one PSUM tile.
                pf = ps_f.tile([A, FA], f32)
                nc.tensor.matmul(pf[:, 0:128], atoms_sb[:, 0:128], etp)
                nc.tensor.matmul(pf[:, 128:256], atoms_sb[:, 128:256], etp)

                featT01 = pfeat.tile([A, FA], f32r)
                nc.scalar.copy(featT01[:], pf[:, 0:FA])

                # Bond sums on DVE, then one transpose matmul -> (fb, a).
                sumbond = pfeat.tile([A, FB], f32r)
                with nc.allow_low_precision(
                        reason="f32r rounding of bond sums"):
                    nc.vector.reduce_sum(
                        sumbond[:],
                        bonds_sb.rearrange("p (d f) -> p f d", d=D),
                        axis=mybir.AxisListType.X)
                pc2 = ps_c2.tile([FB, A], f32)
                nc.tensor.matmul(pc2[:], sumbond[:], identr[:])
                chunk2 = pfeat.tile([FAUG - 256, A], f32r)
                nc.scalar.copy(chunk2[0:FB, :], pc2[:])
                nc.vector.tensor_copy(chunk2[FB:FB + 1, :], ones_row[:])

                # maskdiag_d = diag(deg == d): (I*(deg+1)) == (d+1).
                md = pmd.tile([A, D * A], f32r)
                for d in range(D):
                    nc.vector.tensor_scalar(md[:, d * A:(d + 1) * A],
                                            ident[:], degp1[:], float(d + 1),
                                            OP.mult, OP.is_equal)

                # Dense: Z[:, d*256:(d+1)*256] = feat @ Waug[d].
                lhs = [featT01[:, 0:128], featT01[:, 128:256], chunk2[:]]
                rhs = [w0, w1, w2]
                groups = [(0, 512), (512, 1024), (1024, 1280)]
                zsb = pz.tile([A, D * C], f32r)
                for g0, g1 in groups:
                    pzg = ps_z.tile([A, 512], f32, tag="pzg", bufs=4)
                    nc.tensor.matmul(pzg[:, 0:g1 - g0], lhs[0],
                                     rhs[0][:, g0:g1], start=True, stop=False)
                    nc.tensor.matmul(pzg[:, 0:g1 - g0], lhs[1],
                                     rhs[1][:, g0:g1], start=False,
                                     stop=False)
                    nc.tensor.matmul(pzg[:, 0:g1 - g0], lhs[2],
                                     rhs[2][:, g0:g1], start=False, stop=True)
                    nc.scalar.copy(zsb[:, g0:g1], pzg[:, 0:g1 - g0])

                # Degree select, then one relu into the group output tile.
                pst = ps_s.tile([A, C], f32)
                for d in range(D):
                    nc.tensor.matmul(pst[:], md[:, d * A:(d + 1) * A],
                                     zsb[:, d * C:(d + 1) * C],
                                     start=(d == 0), stop=(d == D - 1))
                nc.scalar.activation(out4[:, j * C:(j + 1) * C], pst[:],
                                     AF.Relu)
            nc.gpsimd.dma_start(
                out=out_ap[bg * G:(bg + 1) * G].rearrange("g p f -> p g f"),
                in_=out4.rearrange("p (g f) -> p g f", g=G))

    nc.compile()
    return nc


def _get_nc():
    if "nc" not in _CACHE:
        _CACHE["nc"] = _build_program()
    return _CACHE["nc"]


def _make_in_maps(atoms, bonds, edges, W, b):
    atoms = np.ascontiguousarray(np.asarray(atoms, dtype=np.float32))
    bonds = np.ascontiguousarray(np.asarray(bonds, dtype=np.float32))
    edges = np.asarray(edges)
    W = np.asarray(W, dtype=np.float32)
    b = np.asarray(b, dtype=np.float32)

    # bf16 edge slots (exact for -1..127) replicated down the partition axis
    # (layout prep for the on-chip one-hot compare; DMA cannot zero-step
    # partitions).
    import ml_dtypes
    edges_f = np.ascontiguousarray(edges.transpose(0, 2, 1)).reshape(
        B, D * A).astype(ml_dtypes.bfloat16)
    edges_rep = np.ascontiguousarray(
        np.broadcast_to(edges_f[:, None, :], (B, A, D * A)))

    waug = np.ascontiguousarray(
        np.concatenate([W, b[:, None, :]], axis=1))           # (5, 321, 256)
    ident = np.eye(A, dtype=np.float32)
    iota = np.arange(A, dtype=np.float32).reshape(A, 1)
    onesr = np.ones((1, A), dtype=np.float32)

    edeg8 = edges.reshape(NCORES, BL, A, D).astype(np.float32)
    atoms8 = atoms.reshape(NCORES, BL, A, FA)
    bonds8 = bonds.reshape(NCORES, BL, A, D * FB)
    edges8 = edges_rep.reshape(NCORES, BL, A, A * D)

    return [
        {
            "atoms": atoms8[c],
            "bonds": bonds8[c],
            "edges": edges8[c],
            "waug": waug,
            "ident": ident,
            "identr": ident,
            "identb": ident.astype(ml_dtypes.bfloat16),
            "iota": iota,
            "edeg": edeg8[c],
            "onesr": onesr,
        }
        for c in range(NCORES)
    ]


def run_sharded(atoms, bonds, edges, W, b, trace=False):
    """Run on the 8 NeuronCores; returns (output, BassKernelResults)."""
    from concourse.bass_utils import run_bass_kernel_spmd

    nc = _get_nc()
    in_maps = _make_in_maps(atoms, bonds, edges, W, b)
    res = run_bass_kernel_spmd(nc, in_maps, list(range(NCORES)), trace=trace)
    out = np.concatenate([res.results[c]["out"] for c in range(NCORES)],
                         axis=0).reshape(B, A, C)
    return out, res


def kernel(atoms, bonds, edges, W, b):
    out, _ = run_sharded(atoms, bonds, edges, W, b)
    return out



# revision 2
# speedup vs baseline: 1.1985x; 1.1985x over previous
"""Trainium2 Bass kernel for NeuralGraphHidden (GNN message passing).

v3 -> v4: the gather contraction is compacted on the host. Only ~21 atoms
per molecule are ever referenced (self + neighbors of active atoms), so the
host stacks each molecule's referenced atom rows and first-fit packs
molecules into 128-row bins (4 bins per compact tile). The per-bin gather
matrix is block-diagonal over molecules, so ONE matmul gathers several
molecules at once: 8 gather matmuls total (vs 32) and ~0.8MB of
atoms+gather inputs (vs 3.2MB). ReLU moved to DVE to avoid the ScalarE
ACT_TABLE_LOAD delaying weight DMA triggers.

See kernel_v2.py docstring for the core algorithm (active-atom compaction).
"""

import sys

sys.path.insert(0, "/opt/trn_rl_repo")

import numpy as np

B, A, D = 256, 128, 5
FA, FB, C = 256, 64, 256
F = FA + FB        # 320
FAUG = F + 1       # 321 (bias row)
NCORES = 8
BL = B // NCORES   # 32 molecules per core
TILES = 2          # compact tiles per core
CAP = 128          # compact slots per tile
NBT = 4            # gather bins per tile (128 stacked atom rows each)
NBINS = TILES * NBT

_CACHE = {}


def _build_program(deg_list):
    from contextlib import ExitStack

    import concourse.bass as bass
    import concourse.tile as tile
    from concourse import bacc, mybir

    f32 = mybir.dt.float32
    OP = mybir.AluOpType
    bf16 = mybir.dt.bfloat16

    ND = len(deg_list)
    NW = ND * C  # dense output width

    nc = bacc.Bacc("TRN2", target_bir_lowering=False, debug=False,
                   num_devices=NCORES)

    atomsr_d = nc.dram_tensor("atomsr", [128, NBINS * FA], bf16,
                              kind="ExternalInput")
    gmatr_d = nc.dram_tensor("gmatr", [128, NBINS * CAP], bf16,
                             kind="ExternalInput")
    bondsrc_d = nc.dram_tensor("bondsrc", [FB, D * TILES * CAP], bf16,
                               kind="ExternalInput")
    w0_d = nc.dram_tensor("w0", [128, NW], bf16, kind="ExternalInput")
    w1_d = nc.dram_tensor("w1", [128, NW], bf16, kind="ExternalInput")
    w2_d = nc.dram_tensor("w2", [FAUG - 256, NW], bf16, kind="ExternalInput")
    dmask_d = nc.dram_tensor("dmask", [CAP, TILES * ND], f32,
                             kind="ExternalInput")
    onesr_d = nc.dram_tensor("onesr", [1, TILES * CAP], bf16,
                             kind="ExternalInput")
    identb_d = nc.dram_tensor("identb", [A, A], bf16, kind="ExternalInput")
    out_d = nc.dram_tensor("out", [CAP, TILES * C], f32,
                           kind="ExternalOutput")

    with tile.TileContext(nc) as tc, ExitStack() as ctx:
        consts = ctx.enter_context(tc.tile_pool(name="consts", bufs=1))
        pfeat = ctx.enter_context(tc.tile_pool(name="pfeat", bufs=2))
        psel = ctx.enter_context(tc.tile_pool(name="psel", bufs=2))
        ps_g = ctx.enter_context(
            tc.tile_pool(name="ps_g", bufs=2, space="PSUM"))
        ps_t = ctx.enter_context(
            tc.tile_pool(name="ps_t", bufs=2, space="PSUM"))
        ps_z = ctx.enter_context(
            tc.tile_pool(name="ps_z", bufs=2, space="PSUM"))

        # ---- input DMAs: per-tile chunks, gather inputs first ------------
        atomsr_sb = consts.tile([128, NBINS * FA], bf16)
        gmatr_sb = consts.tile([128, NBINS * CAP], bf16)
        ha, hg = NBT * FA, NBT * CAP
        for t in range(TILES):
            nc.sync.dma_start(
                out=atomsr_sb[:, t * ha:(t + 1) * ha],
                in_=atomsr_d.ap()[:, t * ha:(t + 1) * ha])
            nc.gpsimd.dma_start(
                out=gmatr_sb[:, t * hg:(t + 1) * hg],
                in_=gmatr_d.ap()[:, t * hg:(t + 1) * hg])

        # weights + consts on the scalar queue
        w0 = consts.tile([128, NW], bf16)
        nc.scalar.dma_start(out=w0[:], in_=w0_d.ap()[:])
        w1 = consts.tile([128, NW], bf16)
        nc.scalar.dma_start(out=w1[:], in_=w1_d.ap()[:])
        w2 = consts.tile([FAUG - 256, NW], bf16)
        nc.scalar.dma_start(out=w2[:], in_=w2_d.ap()[:])
        identb = consts.tile([A, A], bf16)
        nc.gpsimd.dma_start(out=identb[:], in_=identb_d.ap()[:])
        onesr = consts.tile([1, TILES * CAP], bf16)
        nc.scalar.dma_start(out=onesr[:], in_=onesr_d.ap()[:])
        dmask = consts.tile([CAP, TILES * ND], f32)
        nc.scalar.dma_start(out=dmask[:], in_=dmask_d.ap()[:])
        bondsrc = consts.tile([FB, D * TILES * CAP], bf16)
        nc.scalar.dma_start(out=bondsrc[:], in_=bondsrc_d.ap()[:])

        # bond sums per slot: [FB, TILES*CAP] = sum_d bondsrc
        bsum = consts.tile([FB, TILES * CAP], bf16)
        with nc.allow_low_precision(reason="bf16 bond sums, tol 2e-2"):
            nc.vector.reduce_sum(
                bsum[:],
                bondsrc.rearrange("p (d n) -> p n d", d=D),
                axis=mybir.AxisListType.X)

        out_sb = consts.tile([CAP, TILES * C], f32)

        # ---- per-tile pipeline -------------------------------------------
        for t in range(TILES):
            # gather: gath[slots, feat] = sum_bins Gbin^T @ atomsbin
            gp = ps_g.tile([CAP, FA], f32)
            for k in range(NBT):
                bi = t * NBT + k
                nc.tensor.matmul(gp[:],
                                 gmatr_sb[:, bi * CAP:(bi + 1) * CAP],
                                 atomsr_sb[:, bi * FA:(bi + 1) * FA],
                                 start=(k == 0), stop=(k == NBT - 1))
            ga = pfeat.tile([CAP, FA], bf16, tag="ga")
            with nc.allow_low_precision(reason="bf16 feat, tol 2e-2"):
                nc.scalar.copy(ga[:], gp[:])

            # transpose to featT chunks [128 feat, CAP]
            featT = pfeat.tile([128, 2 * CAP], bf16, tag="ftT")
            for k in range(2):
                tp = ps_t.tile([128, CAP], bf16, tag="tp")
                nc.tensor.transpose(tp[:], ga[:, k * 128:(k + 1) * 128],
                                    identb[:])
                nc.scalar.copy(featT[:, k * CAP:(k + 1) * CAP], tp[:])

            # chunk2 = [bond sums; ones] [FB+1, CAP]
            chunk2 = pfeat.tile([FB + 1, CAP], bf16, tag="c2")
            nc.vector.tensor_copy(chunk2[0:FB, :],
                                  bsum[:, t * CAP:(t + 1) * CAP])
            nc.vector.tensor_copy(chunk2[FB:FB + 1, :],
                                  onesr[:, t * CAP:(t + 1) * CAP])

            # dense Z[slots, NW] in PSUM groups of <=512, select on DVE
            acc = psel.tile([CAP, C], f32, tag="acc")
            nacc = 0
            for g0 in range(0, NW, 512):
                g1 = min(g0 + 512, NW)
                pz = ps_z.tile([CAP, 512], f32, tag="pz")
                nc.tensor.matmul(pz[:, 0:g1 - g0], featT[:, 0:CAP],
                                 w0[:, g0:g1], start=True, stop=False)
                nc.tensor.matmul(pz[:, 0:g1 - g0], featT[:, CAP:2 * CAP],
                                 w1[:, g0:g1], start=False, stop=False)
                nc.tensor.matmul(pz[:, 0:g1 - g0], chunk2[:],
                                 w2[:, g0:g1], start=False, stop=True)
                for di in range(g0 // C, (g1 + C - 1) // C):
                    blk = pz[:, di * C - g0:(di + 1) * C - g0]
                    mcol = dmask[:, t * ND + di:t * ND + di + 1]
                    if nacc == 0:
                        nc.vector.tensor_scalar(acc[:], blk, mcol, None,
                                                OP.mult)
                    else:
                        tmp = psel.tile([CAP, C], f32, tag="tmp")
                        nc.vector.tensor_scalar(tmp[:], blk, mcol, None,
                                                OP.mult)
                        nc.vector.tensor_add(acc[:], acc[:], tmp[:])
                    nacc += 1

            # ReLU on DVE (max with 0), straight into the output tile
            nc.vector.tensor_scalar(out_sb[:, t * C:(t + 1) * C], acc[:],
                                    0.0, None, OP.max)

        nc.sync.dma_start(out=out_d.ap()[:], in_=out_sb[:])

    nc.compile()
    return nc


def _get_nc(deg_list):
    key = ("nc", tuple(deg_list))
    if key not in _CACHE:
        _CACHE[key] = _build_program(deg_list)
    return _CACHE[key]


def _prep(atoms, bonds, edges, W, b):
    """Host-side: degree analysis, tile/bin packing, gather matrices."""
    import ml_dtypes

    bf16 = ml_dtypes.bfloat16
    atoms = np.asarray(atoms, dtype=np.float32)
    bonds = np.asarray(bonds, dtype=np.float32)
    edges = np.asarray(edges)
    W = np.asarray(W, dtype=np.float32)
    b = np.asarray(b, dtype=np.float32)

    deg = (edges != -1).sum(axis=-1)                      # (B, A)
    active = deg <= D - 1                                 # (B, A)
    deg_list = sorted(int(d) for d in np.unique(deg[active]))
    if not deg_list:
        deg_list = [D - 1]
    ND = len(deg_list)
    dpos = {d: i for i, d in enumerate(deg_list)}

    waug = np.concatenate([W, b[:, None, :]], axis=1)     # (D, FAUG, C)
    waug = waug[deg_list].astype(bf16)                    # (ND, FAUG, C)
    w0_h = np.ascontiguousarray(
        waug[:, 0:128, :].transpose(1, 0, 2).reshape(128, ND * C))
    w1_h = np.ascontiguousarray(
        waug[:, 128:256, :].transpose(1, 0, 2).reshape(128, ND * C))
    w2_h = np.ascontiguousarray(
        waug[:, 256:FAUG, :].transpose(1, 0, 2).reshape(FAUG - 256, ND * C))

    in_maps = []
    scatter = []   # per core: (slot_flat_idx, mol_global, atom)
    for c in range(NCORES):
        gm0 = c * BL
        # per-molecule active atoms and referenced (self+neighbor) atoms
        acts, refs, refpos = [], [], []
        for m in range(BL):
            gm = gm0 + m
            aidx = np.nonzero(active[gm])[0]
            acts.append(aidx)
            ra = set(aidx.tolist())
            for a in aidx:
                for e in edges[gm, a]:
                    if e >= 0:
                        ra.add(int(e))
            ra = sorted(ra)
            refs.append(ra)
            refpos.append({a: i for i, a in enumerate(ra)})

        counts = np.array([len(x) for x in acts])
        # tiles: first-fit decreasing by slot count, capacity CAP slots
        order = np.argsort(-counts, kind="stable")
        tsum = [0] * TILES
        tmem = [[] for _ in range(TILES)]
        for m in order:
            t = min(range(TILES), key=lambda t: tsum[t])
            if tsum[t] + counts[m] > CAP:
                raise RuntimeError(f"core {c}: slot packing failed")
            tmem[t].append(int(m))
            tsum[t] += int(counts[m])

        atomsr = np.zeros((128, NBINS * FA), dtype=np.float32)
        gmatr = np.zeros((128, NBINS * CAP), dtype=np.float32)
        bondsrc = np.zeros((D, FB, TILES * CAP), dtype=np.float32)
        dmask = np.zeros((CAP, TILES * ND), dtype=np.float32)
        sc_flat, sc_m, sc_a = [], [], []
        for t in range(TILES):
            # slot offsets per molecule in this tile
            offs = {}
            off = 0
            for m in tmem[t]:
                offs[m] = off
                off += int(counts[m])
            # bins: first-fit decreasing by referenced-row count
            rsum = [0] * NBT
            rmem = [[] for _ in range(NBT)]
            for m in sorted(tmem[t], key=lambda m: -len(refs[m])):
                placed = False
                for k in range(NBT):
                    if rsum[k] + len(refs[m]) <= 128:
                        rmem[k].append(m)
                        rsum[k] += len(refs[m])
                        placed = True
                        break
                if not placed:
                    raise RuntimeError(f"core {c}: bin packing failed")
            for k in range(NBT):
                bi = t * NBT + k
                r = 0
                for m in rmem[k]:
                    gm = gm0 + m
                    ra = refs[m]
                    L = len(ra)
                    atomsr[r:r + L, bi * FA:(bi + 1) * FA] = atoms[gm, ra]
                    for i, a in enumerate(acts[m]):
                        col = offs[m] + i
                        gmatr[r + refpos[m][a], bi * CAP + col] += 1.0
                        for dd in range(D):
                            e = edges[gm, a, dd]
                            if e >= 0:
                                gmatr[r + refpos[m][int(e)],
                                      bi * CAP + col] += 1.0
                        bondsrc[:, :, t * CAP + col] = bonds[gm, a]
                        dmask[col, t * ND + dpos[int(deg[gm, a])]] = 1.0
                        sc_flat.append(t * CAP + col)
                        sc_m.append(gm)
                        sc_a.append(int(a))
                    r += L
        scatter.append((np.asarray(sc_flat), np.asarray(sc_m),
                        np.asarray(sc_a)))
        in_maps.append({
            "atomsr": atomsr.astype(bf16),
            "gmatr": gmatr.astype(bf16),
            "bondsrc": np.ascontiguousarray(
                bondsrc.astype(bf16).transpose(1, 0, 2).reshape(
                    FB, D * TILES * CAP)),
            "w0": w0_h,
            "w1": w1_h,
            "w2": w2_h,
            "dmask": dmask,
            "onesr": np.ones((1, TILES * CAP), dtype=bf16),
            "identb": np.eye(A, dtype=np.float32).astype(bf16),
        })
    return deg_list, in_maps, scatter


def run_sharded(atoms, bonds, edges, W, b, trace=False):
    """Run on the 8 NeuronCores; returns (output, BassKernelResults)."""
    from concourse.bass_utils import run_bass_kernel_spmd

    deg_list, in_maps, scatter = _prep(atoms, bonds, edges, W, b)
    nc = _get_nc(deg_list)
    res = run_bass_kernel_spmd(nc, in_maps, list(range(NCORES)), trace=trace)
    out = np.zeros((B, A, C), dtype=np.float32)
    for c in range(NCORES):
        sc_flat, sc_m, sc_a = scatter[c]
        oc = res.results[c]["out"]
        t_idx = sc_flat // CAP
        s_idx = sc_flat % CAP
        out[sc_m, sc_a] = oc[s_idx[:, None], (t_idx[:, None] * C +
                                              np.arange(C)[None, :])]
    return out, res


def kernel(atoms, bonds, edges, W, b):
    out, _ = run_sharded(atoms, bonds, edges, W, b)
    return out


# revision 3
# speedup vs baseline: 1.3457x; 1.1228x over previous
"""Trainium2 Bass kernel for NeuralGraphHidden (GNN message passing), v5.

v4 -> v5:
  - gather matmuls write featT layout directly (lhs = atom-row chunks), so
    the PE transposes, identity matrix and extra PSUM->SBUF copy disappear.
  - all small constants (W chunks, bond rows, degree masks) ship as ONE
    contiguous blob DMA; per-tile atom-rows+gather-matrix ship as one blob
    per tile. Fewer DMAs -> far less descriptor-generation serialization.
  - tile 0 holds only slots of the modal degree (4): its dense runs just
    that one weight block and needs no select masks at all.
  - per-tile output DMAs on a dedicated queue overlap tile-1 compute.
  - ~26 dummy matmuls on a zero tile run during the input-DMA wait to warm
    the PE HAM clock gate from 1.2 GHz to 2.4 GHz before real work.

See kernel_v2.py docstring for the core algorithm (active-atom compaction).
"""

import sys

sys.path.insert(0, "/opt/trn_rl_repo")

import numpy as np

B, A, D = 256, 128, 5
FA, FB, C = 256, 64, 256
F = FA + FB        # 320
FAUG = F + 1       # 321 (bias row)
NCORES = 8
BL = B // NCORES   # 32 molecules per core
TILES = 2          # compact tiles per core
CAP = 128          # compact slots per tile
BINW = FA + CAP    # per-bin blob width (256 atom cols + 128 gather cols)
NWARM = 26         # PE clock-gate warmup matmuls

_CACHE = {}


def _build_program(deg_list, pure_di, nbt):
    from contextlib import ExitStack

    import concourse.bass as bass
    import concourse.tile as tile
    from concourse import bacc, mybir

    f32 = mybir.dt.float32
    OP = mybir.AluOpType
    bf16 = mybir.dt.bfloat16

    ND = len(deg_list)
    NW = ND * C
    TW = nbt * BINW          # per-tile input blob width
    # consts blob columns: w0 | w1 | w2 | bondsrc | dmask
    CB_W0, CB_W1, CB_W2 = 0, NW, 2 * NW
    CB_BD = 3 * NW
    CB_DM = CB_BD + D * TILES * CAP
    CBW = CB_DM + TILES * ND

    nc = bacc.Bacc("TRN2", target_bir_lowering=False, debug=False,
                   num_devices=NCORES)

    inblob_d = nc.dram_tensor("inblob", [128, TILES * TW], bf16,
                              kind="ExternalInput")
    cblob_d = nc.dram_tensor("cblob", [128, CBW], bf16,
                             kind="ExternalInput")
    out_d = nc.dram_tensor("out", [CAP, TILES * C], f32,
                           kind="ExternalOutput")

    with tile.TileContext(nc) as tc, ExitStack() as ctx:
        consts = ctx.enter_context(tc.tile_pool(name="consts", bufs=1))
        pfeat = ctx.enter_context(tc.tile_pool(name="pfeat", bufs=2))
        psel = ctx.enter_context(tc.tile_pool(name="psel", bufs=2))
        ps_f = ctx.enter_context(
            tc.tile_pool(name="ps_f", bufs=2, space="PSUM"))
        ps_z = ctx.enter_context(
            tc.tile_pool(name="ps_z", bufs=2, space="PSUM"))

        # ---- PE warmup: dummy matmuls on a zeroed tile -------------------
        warm0 = consts.tile([128, 128], bf16)
        nc.vector.memset(warm0[:], 0.0)
        for i in range(NWARM):
            pw = ps_z.tile([128, 128], f32, tag="pw")
            nc.tensor.matmul(pw[:], warm0[:], warm0[:], start=True, stop=True)

        # ---- input DMAs --------------------------------------------------
        inblob = consts.tile([128, TILES * TW], bf16)
        for t in range(TILES):
            nc.sync.dma_start(out=inblob[:, t * TW:(t + 1) * TW],
                              in_=inblob_d.ap()[:, t * TW:(t + 1) * TW])
        cblob = consts.tile([128, CBW], bf16)
        nc.scalar.dma_start(out=cblob[:], in_=cblob_d.ap()[:])

        w0 = cblob[:, CB_W0:CB_W0 + NW]
        w1 = cblob[:, CB_W1:CB_W1 + NW]
        w2 = cblob[0:FAUG - 256, CB_W2:CB_W2 + NW]
        bond = cblob[0:FB, CB_BD:CB_BD + D * TILES * CAP]
        dmaskb = cblob[:, CB_DM:CB_DM + TILES * ND]

        dmask = consts.tile([CAP, TILES * ND], f32)
        nc.vector.tensor_copy(dmask[:], dmaskb)

        # bond sums per slot (split per tile so tile 0 unblocks early)
        bsum = consts.tile([FB, TILES * CAP], bf16)
        vb = bond.rearrange("p (d n) -> p n d", d=D)
        with nc.allow_low_precision(reason="bf16 bond sums, tol 2e-2"):
            for t in range(TILES):
                nc.vector.reduce_sum(
                    bsum[:, t * CAP:(t + 1) * CAP],
                    vb[:, t * CAP:(t + 1) * CAP, :],
                    axis=mybir.AxisListType.X)

        out_sb = consts.tile([CAP, TILES * C], f32)

        # ---- per-tile pipeline -------------------------------------------
        for t in range(TILES):
            base = t * TW
            # gather straight into featT layout: [feat chunk, slots]
            fps = []
            for k in range(2):
                fp = ps_f.tile([128, CAP], f32, tag=f"fp{k}")
                for bi in range(nbt):
                    nc.tensor.matmul(
                        fp[:],
                        inblob[:, base + bi * BINW + k * 128:
                               base + bi * BINW + (k + 1) * 128],
                        inblob[:, base + bi * BINW + FA:
                               base + (bi + 1) * BINW],
                        start=(bi == 0), stop=(bi == nbt - 1))
                fps.append(fp)
            featT = pfeat.tile([128, 2 * CAP], bf16, tag="ftT")
            with nc.allow_low_precision(reason="bf16 feat, tol 2e-2"):
                for k in range(2):
                    nc.scalar.copy(featT[:, k * CAP:(k + 1) * CAP],
                                   fps[k][:])

            # chunk2 = [bond sums; ones] [FB+1, CAP]
            chunk2 = pfeat.tile([FB + 1, CAP], bf16, tag="c2")
            nc.vector.tensor_copy(chunk2[0:FB, :],
                                  bsum[:, t * CAP:(t + 1) * CAP])
            nc.vector.memset(chunk2[FB:FB + 1, :], 1.0)

            if t == 0 and pure_di is not None:
                # pure-modal-degree tile: single weight block, no select
                pz = ps_z.tile([CAP, 512], f32, tag="pz")
                g0 = pure_di * C
                nc.tensor.matmul(pz[:, 0:C], featT[:, 0:CAP],
                                 w0[:, g0:g0 + C], start=True, stop=False)
                nc.tensor.matmul(pz[:, 0:C], featT[:, CAP:2 * CAP],
                                 w1[:, g0:g0 + C], start=False, stop=False)
                nc.tensor.matmul(pz[:, 0:C], chunk2[:],
                                 w2[:, g0:g0 + C], start=False, stop=True)
                nc.vector.tensor_scalar(out_sb[:, t * C:(t + 1) * C],
                                        pz[:, 0:C], 0.0, None, OP.max)
            else:
                acc = psel.tile([CAP, C], f32, tag="acc")
                nacc = 0
                for g0 in range(0, NW, 512):
                    g1 = min(g0 + 512, NW)
                    pz = ps_z.tile([CAP, 512], f32, tag="pz")
                    nc.tensor.matmul(pz[:, 0:g1 - g0], featT[:, 0:CAP],
                                     w0[:, g0:g1], start=True, stop=False)
                    nc.tensor.matmul(pz[:, 0:g1 - g0],
                                     featT[:, CAP:2 * CAP],
                                     w1[:, g0:g1], start=False, stop=False)
                    nc.tensor.matmul(pz[:, 0:g1 - g0], chunk2[:],
                                     w2[:, g0:g1], start=False, stop=True)
                    for di in range(g0 // C, (g1 + C - 1) // C):
                        blk = pz[:, di * C - g0:(di + 1) * C - g0]
                        mcol = dmask[:, t * ND + di:t * ND + di + 1]
                        if nacc == 0:
                            nc.vector.tensor_scalar(acc[:], blk, mcol,
                                                    None, OP.mult)
                        else:
                            tmp = psel.tile([CAP, C], f32, tag="tmp")
                            nc.vector.tensor_scalar(tmp[:], blk, mcol,
                                                    None, OP.mult)
                            nc.vector.tensor_add(acc[:], acc[:], tmp[:])
                        nacc += 1
                nc.vector.tensor_scalar(out_sb[:, t * C:(t + 1) * C],
                                        acc[:], 0.0, None, OP.max)

            # per-tile output DMA on its own queue, overlaps next tile
            nc.gpsimd.dma_start(out=out_d.ap()[:, t * C:(t + 1) * C],
                                in_=out_sb[:, t * C:(t + 1) * C])

    nc.compile()
    return nc


def _get_nc(deg_list, pure_di, nbt):
    key = ("nc", tuple(deg_list), pure_di, nbt)
    if key not in _CACHE:
        _CACHE[key] = _build_program(deg_list, pure_di, nbt)
    return _CACHE[key]


def _prep(atoms, bonds, edges, W, b):
    """Host-side: degree analysis, tile/bin packing, blob assembly."""
    import ml_dtypes

    bf16 = ml_dtypes.bfloat16
    atoms = np.asarray(atoms, dtype=np.float32)
    bonds = np.asarray(bonds, dtype=np.float32)
    edges = np.asarray(edges)
    W = np.asarray(W, dtype=np.float32)
    b = np.asarray(b, dtype=np.float32)

    deg = (edges != -1).sum(axis=-1)                      # (B, A)
    active = deg <= D - 1                                 # (B, A)
    deg_list = sorted(int(d) for d in np.unique(deg[active]))
    if not deg_list:
        deg_list = [D - 1]
    ND = len(deg_list)
    dpos = {d: i for i, d in enumerate(deg_list)}
    # modal degree (most common among active atoms) for the pure tile
    dcounts = {d: int((deg[active] == d).sum()) for d in deg_list}
    dmod = max(deg_list, key=lambda d: dcounts[d])

    waug = np.concatenate([W, b[:, None, :]], axis=1)     # (D, FAUG, C)
    waug = waug[deg_list].astype(bf16)                    # (ND, FAUG, C)
    NW = ND * C

    # ---- per-core packing --------------------------------------------
    cores = []
    nbt_req = 4
    pure_ok = True
    for c in range(NCORES):
        gm0 = c * BL
        acts, refs, refpos, pure = [], [], [], []
        for m in range(BL):
            gm = gm0 + m
            aidx = np.nonzero(active[gm])[0]
            acts.append(aidx)
            ra = set(aidx.tolist())
            for a in aidx:
                for e in edges[gm, a]:
                    if e >= 0:
                        ra.add(int(e))
            ra = sorted(ra)
            refs.append(ra)
            refpos.append({a: i for i, a in enumerate(ra)})
            pure.append(all(deg[gm, a] == dmod for a in aidx))

        slots = [0, 0]
        rsums = [0, 0]
        tmem = [[], []]

        def fits(t, m):
            return (slots[t] + len(acts[m]) <= CAP and
                    rsums[t] + len(refs[m]) <= 460)

        ok = True
        order = sorted(range(BL), key=lambda m: -len(refs[m]))
        for m in order:
            if len(acts[m]) == 0:
                continue
            if not pure[m]:
                if fits(1, m):
                    t = 1
                else:
                    ok = False
                    break
            else:
                cands = [t for t in range(TILES) if fits(t, m)]
                if not cands:
                    ok = False
                    break
                t = min(cands, key=lambda t: rsums[t])
            tmem[t].append(m)
            slots[t] += len(acts[m])
            rsums[t] += len(refs[m])
        if not ok:
            pure_ok = False
        cores.append((acts, refs, refpos, tmem))

    if not pure_ok:
        # fallback: balanced packing, select on both tiles
        cores = []
        for c in range(NCORES):
            gm0 = c * BL
            acts, refs, refpos = [], [], []
            for m in range(BL):
                gm = gm0 + m
                aidx = np.nonzero(active[gm])[0]
                acts.append(aidx)
                ra = set(aidx.tolist())
                for a in aidx:
                    for e in edges[gm, a]:
                        if e >= 0:
                            ra.add(int(e))
                ra = sorted(ra)
                refs.append(ra)
                refpos.append({a: i for i, a in enumerate(ra)})
            slots = [0, 0]
            rsums = [0, 0]
            tmem = [[], []]
            for m in sorted(range(BL), key=lambda m: -len(refs[m])):
                if len(acts[m]) == 0:
                    continue
                cands = [t for t in range(TILES)
                         if slots[t] + len(acts[m]) <= CAP]
                t = min(cands, key=lambda t: rsums[t])
                tmem[t].append(m)
                slots[t] += len(acts[m])
                rsums[t] += len(refs[m])
            cores.append((acts, refs, refpos, tmem))

    # bins per tile (first-fit decreasing); find the max bin count needed
    binned = []
    for c in range(NCORES):
        acts, refs, refpos, tmem = cores[c]
        tb = []
        for t in range(TILES):
            bins = []   # list of (rows_used, [mols])
            for m in sorted(tmem[t], key=lambda m: -len(refs[m])):
                for bn in bins:
                    if bn[0] + len(refs[m]) <= 128:
                        bn[0] += len(refs[m])
                        bn[1].append(m)
                        break
                else:
                    bins.append([len(refs[m]), [m]])
            tb.append([bn[1] for bn in bins])
            nbt_req = max(nbt_req, len(bins))
        binned.append(tb)
    nbt = nbt_req
    TW = nbt * BINW
    pure_di = dpos[dmod] if pure_ok else None

    CB_W0, CB_W1, CB_W2 = 0, NW, 2 * NW
    CB_BD = 3 * NW
    CB_DM = CB_BD + D * TILES * CAP
    CBW = CB_DM + TILES * ND

    in_maps = []
    scatter = []
    for c in range(NCORES):
        gm0 = c * BL
        acts, refs, refpos, tmem = cores[c]
        inblob = np.zeros((128, TILES * TW), dtype=np.float32)
        cblob = np.zeros((128, CBW), dtype=np.float32)
        cblob[0:128, CB_W0:CB_W0 + NW] = \
            waug[:, 0:128, :].transpose(1, 0, 2).reshape(128, NW)
        cblob[0:128, CB_W1:CB_W1 + NW] = \
            waug[:, 128:256, :].transpose(1, 0, 2).reshape(128, NW)
        cblob[0:FAUG - 256, CB_W2:CB_W2 + NW] = \
            waug[:, 256:FAUG, :].transpose(1, 0, 2).reshape(FAUG - 256, NW)
        bondsrc = np.zeros((D, FB, TILES * CAP), dtype=np.float32)
        dmask = np.zeros((CAP, TILES * ND), dtype=np.float32)
        sc_flat, sc_m, sc_a = [], [], []
        for t in range(TILES):
            offs = {}
            off = 0
            for m in tmem[t]:
                offs[m] = off
                off += len(acts[m])
            for k, mols in enumerate(binned[c][t]):
                base = t * TW + k * BINW
                r = 0
                for m in mols:
                    gm = gm0 + m
                    ra = refs[m]
                    L = len(ra)
                    inblob[r:r + L, base:base + FA] = atoms[gm, ra]
                    for i, a in enumerate(acts[m]):
                        col = offs[m] + i
                        inblob[r + refpos[m][a], base + FA + col] += 1.0
                        for dd in range(D):
                            e = edges[gm, a, dd]
                            if e >= 0:
                                inblob[r + refpos[m][int(e)],
                                       base + FA + col] += 1.0
                        bondsrc[:, :, t * CAP + col] = bonds[gm, a]
                        dmask[col, t * ND + dpos[int(deg[gm, a])]] = 1.0
                        sc_flat.append(t * CAP + col)
                        sc_m.append(gm)
                        sc_a.append(int(a))
                    r += L
        cblob[0:FB, CB_BD:CB_BD + D * TILES * CAP] = \
            bondsrc.transpose(1, 0, 2).reshape(FB, D * TILES * CAP)
        cblob[0:CAP, CB_DM:CB_DM + TILES * ND] = dmask
        scatter.append((np.asarray(sc_flat), np.asarray(sc_m),
                        np.asarray(sc_a)))
        in_maps.append({
            "inblob": inblob.astype(bf16),
            "cblob": cblob.astype(bf16),
        })
    return deg_list, pure_di, nbt, in_maps, scatter


def run_sharded(atoms, bonds, edges, W, b, trace=False):
    """Run on the 8 NeuronCores; returns (output, BassKernelResults)."""
    from concourse.bass_utils import run_bass_kernel_spmd

    deg_list, pure_di, nbt, in_maps, scatter = _prep(atoms, bonds, edges,
                                                     W, b)
    nc = _get_nc(deg_list, pure_di, nbt)
    res = run_bass_kernel_spmd(nc, in_maps, list(range(NCORES)), trace=trace)
    out = np.zeros((B, A, C), dtype=np.float32)
    for c in range(NCORES):
        sc_flat, sc_m, sc_a = scatter[c]
        oc = res.results[c]["out"]
        t_idx = sc_flat // CAP
        s_idx = sc_flat % CAP
        out[sc_m, sc_a] = oc[s_idx[:, None], (t_idx[:, None] * C +
                                              np.arange(C)[None, :])]
    return out, res


def kernel(atoms, bonds, edges, W, b):
    out, _ = run_sharded(atoms, bonds, edges, W, b)
    return out


# revision 4
# speedup vs baseline: 1.3929x; 1.0351x over previous
"""Trainium2 Bass kernel for NeuralGraphHidden (GNN message passing), v6.

v5 -> v6: degree select is eliminated entirely. Within each compact tile,
slot columns are grouped by atom degree into FIXED ranges (capacities =
max needed across cores, so the SPMD program is core-independent). The
dense stage runs one 3-matmul chain per non-empty range against just that
degree's weight block, so no masks, no DVE select chain, and tile 0
(packed with pure-modal-degree molecules) is a single range. Support DMAs
are split per consumer (bond rows ship as their own 64-descriptor tensor),
bond reduces + chunk2 assembly run on GpSimd, featT copies on Vector, so
ScalarE only triggers weight DMAs (no ACT_TABLE_LOAD stall).

See kernel_v2.py docstring for the core algorithm (active-atom compaction).
"""

import sys

sys.path.insert(0, "/opt/trn_rl_repo")

import numpy as np

B, A, D = 256, 128, 5
FA, FB, C = 256, 64, 256
F = FA + FB        # 320
FAUG = F + 1       # 321 (bias row)
NCORES = 8
BL = B // NCORES   # 32 molecules per core
TILES = 2          # compact tiles per core
CAP = 128          # compact slots per tile
BINW = FA + CAP    # per-bin blob width (256 atom cols + 128 gather cols)
NWARM = 26         # PE clock-gate warmup matmuls

_CACHE = {}


def _build_program(ranges, nbts):
    """ranges: per tile, tuple of (base, width, weight_block_index).
    nbts: gather bin count per tile."""
    from contextlib import ExitStack

    import concourse.bass as bass
    import concourse.tile as tile
    from concourse import bacc, mybir

    f32 = mybir.dt.float32
    OP = mybir.AluOpType
    bf16 = mybir.dt.bfloat16

    ND = max(di for tr in ranges for (_, _, di) in tr) + 1
    NW = ND * C
    tbase = [0]
    for t in range(TILES):
        tbase.append(tbase[-1] + nbts[t] * BINW)
    IBW = tbase[-1]          # total input blob width

    nc = bacc.Bacc("TRN2", target_bir_lowering=False, debug=False,
                   num_devices=NCORES)

    inblob_d = nc.dram_tensor("inblob", [128, IBW], bf16,
                              kind="ExternalInput")
    wblob_d = nc.dram_tensor("wblob", [128, 3 * NW], bf16,
                             kind="ExternalInput")
    bond_d = nc.dram_tensor("bond", [FB, D * TILES * CAP], bf16,
                            kind="ExternalInput")
    out_d = nc.dram_tensor("out", [CAP, TILES * C], f32,
                           kind="ExternalOutput")

    with tile.TileContext(nc) as tc, ExitStack() as ctx:
        consts = ctx.enter_context(tc.tile_pool(name="consts", bufs=1))
        pfeat = ctx.enter_context(tc.tile_pool(name="pfeat", bufs=2))
        ps_f = ctx.enter_context(
            tc.tile_pool(name="ps_f", bufs=2, space="PSUM"))
        ps_z = ctx.enter_context(
            tc.tile_pool(name="ps_z", bufs=2, space="PSUM"))

        # ---- PE warmup: dummy matmuls on a zeroed tile -------------------
        warm0 = consts.tile([128, 128], bf16)
        nc.vector.memset(warm0[:], 0.0)
        for i in range(NWARM):
            pw = ps_z.tile([128, 128], f32, tag="pw")
            nc.tensor.matmul(pw[:], warm0[:], warm0[:], start=True, stop=True)

        # ---- input DMAs --------------------------------------------------
        inblob = consts.tile([128, IBW], bf16)
        for t in range(TILES):
            tw = nbts[t] * BINW
            hw = (nbts[t] // 2) * BINW if nbts[t] > 1 else tw
            for lo, hi in ([(tbase[t], tbase[t] + hw),
                            (tbase[t] + hw, tbase[t] + tw)]
                           if hw < tw else [(tbase[t], tbase[t] + tw)]):
                nc.sync.dma_start(out=inblob[:, lo:hi],
                                  in_=inblob_d.ap()[:, lo:hi])
        bond = consts.tile([FB, D * TILES * CAP], bf16)
        nc.scalar.dma_start(out=bond[:], in_=bond_d.ap()[:])
        wblob = consts.tile([128, 3 * NW], bf16)
        nc.scalar.dma_start(out=wblob[:], in_=wblob_d.ap()[:])
        w0 = wblob[:, 0:NW]
        w1 = wblob[:, NW:2 * NW]
        w2 = wblob[0:FAUG - 256, 2 * NW:3 * NW]

        # bond sums per slot on GpSimd (split per tile)
        bsum = consts.tile([FB, TILES * CAP], bf16)
        vb = bond.rearrange("p (d n) -> p n d", d=D)
        with nc.allow_low_precision(reason="bf16 bond sums, tol 2e-2"):
            for t in range(TILES):
                nc.vector.reduce_sum(
                    bsum[:, t * CAP:(t + 1) * CAP],
                    vb[:, t * CAP:(t + 1) * CAP, :],
                    axis=mybir.AxisListType.X)

        out_sb = consts.tile([CAP, TILES * C], f32)
        nc.gpsimd.memset(out_sb[:], 0.0)

        # ---- per-tile pipeline -------------------------------------------
        for t in range(TILES):
            tb = tbase[t]
            # gather straight into featT layout: [feat chunk, slots]
            fps = []
            for k in range(2):
                fp = ps_f.tile([128, CAP], f32, tag=f"fp{k}")
                for bi in range(nbts[t]):
                    nc.tensor.matmul(
                        fp[:],
                        inblob[:, tb + bi * BINW + k * 128:
                               tb + bi * BINW + (k + 1) * 128],
                        inblob[:, tb + bi * BINW + FA:
                               tb + (bi + 1) * BINW],
                        start=(bi == 0), stop=(bi == nbts[t] - 1))
                fps.append(fp)
            featT = pfeat.tile([128, 2 * CAP], bf16, tag="ftT")
            with nc.allow_low_precision(reason="bf16 feat, tol 2e-2"):
                for k in range(2):
                    nc.vector.tensor_copy(featT[:, k * CAP:(k + 1) * CAP],
                                          fps[k][:])

            # chunk2 = [bond sums; ones] [FB+1, CAP] on GpSimd
            chunk2 = pfeat.tile([FB + 1, CAP], bf16, tag="c2")
            nc.gpsimd.tensor_copy(chunk2[0:FB, :],
                                  bsum[:, t * CAP:(t + 1) * CAP])
            nc.gpsimd.memset(chunk2[FB:FB + 1, :], 1.0)

            # dense: one 3-matmul chain per degree range of this tile
            pz = ps_z.tile([CAP, 512], f32, tag="pz")
            for (r0, rw, di) in ranges[t]:
                g0 = di * C
                nc.tensor.matmul(pz[r0:r0 + rw, 0:C],
                                 featT[:, r0:r0 + rw],
                                 w0[:, g0:g0 + C], start=True, stop=False)
                nc.tensor.matmul(pz[r0:r0 + rw, 0:C],
                                 featT[:, CAP + r0:CAP + r0 + rw],
                                 w1[:, g0:g0 + C], start=False, stop=False)
                nc.tensor.matmul(pz[r0:r0 + rw, 0:C],
                                 chunk2[:, r0:r0 + rw],
                                 w2[:, g0:g0 + C], start=False, stop=True)
            tot = ranges[t][-1][0] + ranges[t][-1][1]
            nc.vector.tensor_scalar(out_sb[0:tot, t * C:(t + 1) * C],
                                    pz[0:tot, 0:C], 0.0, None, OP.max)
            nc.gpsimd.dma_start(out=out_d.ap()[:, t * C:(t + 1) * C],
                                in_=out_sb[:, t * C:(t + 1) * C])

    nc.compile()
    return nc


def _get_nc(ranges, nbts):
    key = ("nc", tuple(tuple(tr) for tr in ranges), tuple(nbts))
    if key not in _CACHE:
        _CACHE[key] = _build_program(ranges, nbts)
    return _CACHE[key]


def _prep(atoms, bonds, edges, W, b):
    """Host-side: degree analysis, tile/bin packing, blob assembly."""
    import ml_dtypes

    bf16 = ml_dtypes.bfloat16
    atoms = np.asarray(atoms, dtype=np.float32)
    bonds = np.asarray(bonds, dtype=np.float32)
    edges = np.asarray(edges)
    W = np.asarray(W, dtype=np.float32)
    b = np.asarray(b, dtype=np.float32)

    deg = (edges != -1).sum(axis=-1)                      # (B, A)
    active = deg <= D - 1                                 # (B, A)
    deg_list = sorted(int(d) for d in np.unique(deg[active]))
    if not deg_list:
        deg_list = [D - 1]
    ND = len(deg_list)
    dpos = {d: i for i, d in enumerate(deg_list)}
    dcounts = {d: int((deg[active] == d).sum()) for d in deg_list}
    dmod = max(deg_list, key=lambda d: dcounts[d])

    waug = np.concatenate([W, b[:, None, :]], axis=1)     # (D, FAUG, C)
    waug = waug[deg_list].astype(bf16)                    # (ND, FAUG, C)
    NW = ND * C

    # ---- pass 1: per-core packing ------------------------------------
    cores = []
    need = np.zeros((NCORES, TILES, ND), dtype=int)  # slots per (t, deg)
    for c in range(NCORES):
        gm0 = c * BL
        acts, refs, refpos, pure = [], [], [], []
        for m in range(BL):
            gm = gm0 + m
            aidx = np.nonzero(active[gm])[0]
            acts.append(aidx)
            ra = set(aidx.tolist())
            for a in aidx:
                for e in edges[gm, a]:
                    if e >= 0:
                        ra.add(int(e))
            ra = sorted(ra)
            refs.append(ra)
            refpos.append({a: i for i, a in enumerate(ra)})
            pure.append(all(deg[gm, a] == dmod for a in aidx))

        slots = [0, 0]
        rsums = [0, 0]
        tmem = [[], []]

        def fits(t, m):
            return (slots[t] + len(acts[m]) <= CAP and
                    rsums[t] + len(refs[m]) <= 580)

        order = sorted(range(BL), key=lambda m: -len(refs[m]))
        for m in order:
            if len(acts[m]) == 0:
                continue
            if not pure[m]:
                if not fits(1, m):
                    raise RuntimeError(f"core {c}: tile packing failed")
                t = 1
            else:
                # fill tile 0 first so tile 1 keeps few slots per degree
                if fits(0, m):
                    t = 0
                elif fits(1, m):
                    t = 1
                else:
                    raise RuntimeError(f"core {c}: tile packing failed")
            tmem[t].append(m)
            slots[t] += len(acts[m])
            rsums[t] += len(refs[m])
            for a in acts[m]:
                need[c, t, dpos[int(deg[gm0 + m, a])]] += 1
        cores.append((acts, refs, refpos, tmem))

    # fixed per-(tile, degree) range capacities, shared across cores
    # PE matmul output BASE partitions may only be 0, 32 or 64. Place
    # small ranges first at 0/32, the big (modal) range last at the next
    # free base with room to grow to 128.
    caps = need.max(axis=0)                               # (TILES, ND)
    ranges = []
    for t in range(TILES):
        present = [di for di in range(ND) if caps[t, di] > 0]
        present.sort(key=lambda di: (int(caps[t, di]), deg_list[di]))
        assert len(present) <= 3, "too many degree ranges per tile"
        bases = [0, 32, 64][:len(present)]
        if present:
            bases[-1] = min(64, bases[-1])
        tr = []
        for i, di in enumerate(present):
            b0 = bases[i]
            lim = bases[i + 1] if i + 1 < len(present) else CAP
            w = int(caps[t, di])
            assert b0 + w <= lim, "range capacity overflow"
            tr.append((b0, w, di))
        # big range may span from its base to CAP
        ranges.append(tuple(tr))

    # bins per tile (first-fit decreasing); per-tile bin counts
    binned = []
    nbts = [1] * TILES
    for c in range(NCORES):
        acts, refs, refpos, tmem = cores[c]
        tb = []
        for t in range(TILES):
            bins = []
            for m in sorted(tmem[t], key=lambda m: -len(refs[m])):
                for bn in bins:
                    if bn[0] + len(refs[m]) <= 128:
                        bn[0] += len(refs[m])
                        bn[1].append(m)
                        break
                else:
                    bins.append([len(refs[m]), [m]])
            tb.append([bn[1] for bn in bins])
            nbts[t] = max(nbts[t], len(bins))
        binned.append(tb)
    tbase = [0]
    for t in range(TILES):
        tbase.append(tbase[-1] + nbts[t] * BINW)
    IBW = tbase[-1]

    rbase = {}  # (t, di) -> column base
    for t in range(TILES):
        for (b0, w, di) in ranges[t]:
            rbase[(t, di)] = b0

    in_maps = []
    scatter = []
    for c in range(NCORES):
        gm0 = c * BL
        acts, refs, refpos, tmem = cores[c]
        inblob = np.zeros((128, IBW), dtype=np.float32)
        bondsrc = np.zeros((D, FB, TILES * CAP), dtype=np.float32)
        sc_flat, sc_m, sc_a = [], [], []
        for t in range(TILES):
            cur = {di: 0 for di in range(ND)}
            for k, mols in enumerate(binned[c][t]):
                base = tbase[t] + k * BINW
                r = 0
                for m in mols:
                    gm = gm0 + m
                    ra = refs[m]
                    L = len(ra)
                    inblob[r:r + L, base:base + FA] = atoms[gm, ra]
                    for a in acts[m]:
                        di = dpos[int(deg[gm, a])]
                        col = rbase[(t, di)] + cur[di]
                        cur[di] += 1
                        inblob[r + refpos[m][a], base + FA + col] += 1.0
                        for dd in range(D):
                            e = edges[gm, a, dd]
                            if e >= 0:
                                inblob[r + refpos[m][int(e)],
                                       base + FA + col] += 1.0
                        bondsrc[:, :, t * CAP + col] = bonds[gm, a]
                        sc_flat.append(t * CAP + col)
                        sc_m.append(gm)
                        sc_a.append(int(a))
                    r += L
        wblob = np.zeros((128, 3 * NW), dtype=np.float32)
        wblob[0:128, 0:NW] = \
            waug[:, 0:128, :].transpose(1, 0, 2).reshape(128, NW)
        wblob[0:128, NW:2 * NW] = \
            waug[:, 128:256, :].transpose(1, 0, 2).reshape(128, NW)
        wblob[0:FAUG - 256, 2 * NW:3 * NW] = \
            waug[:, 256:FAUG, :].transpose(1, 0, 2).reshape(FAUG - 256, NW)
        scatter.append((np.asarray(sc_flat), np.asarray(sc_m),
                        np.asarray(sc_a)))
        in_maps.append({
            "inblob": inblob.astype(bf16),
            "wblob": wblob.astype(bf16),
            "bond": np.ascontiguousarray(
                bondsrc.astype(bf16).transpose(1, 0, 2).reshape(
                    FB, D * TILES * CAP)),
        })
    return ranges, nbts, in_maps, scatter


def run_sharded(atoms, bonds, edges, W, b, trace=False):
    """Run on the 8 NeuronCores; returns (output, BassKernelResults)."""
    from concourse.bass_utils import run_bass_kernel_spmd

    ranges, nbts, in_maps, scatter = _prep(atoms, bonds, edges, W, b)
    nc = _get_nc(ranges, nbts)
    res = run_bass_kernel_spmd(nc, in_maps, list(range(NCORES)), trace=trace)
    out = np.zeros((B, A, C), dtype=np.float32)
    for c in range(NCORES):
        sc_flat, sc_m, sc_a = scatter[c]
        oc = res.results[c]["out"]
        t_idx = sc_flat // CAP
        s_idx = sc_flat % CAP
        out[sc_m, sc_a] = oc[s_idx[:, None], (t_idx[:, None] * C +
                                              np.arange(C)[None, :])]
    return out, res


def kernel(atoms, bonds, edges, W, b):
    out, _ = run_sharded(atoms, bonds, edges, W, b)
    return out


# revision 5
# speedup vs baseline: 1.4633x; 1.0505x over previous
"""Trainium2 Bass kernel for NeuralGraphHidden (GNN message passing), v6.

v5 -> v6: degree select is eliminated entirely. Within each compact tile,
slot columns are grouped by atom degree into FIXED ranges (capacities =
max needed across cores, so the SPMD program is core-independent). The
dense stage runs one 3-matmul chain per non-empty range against just that
degree's weight block, so no masks, no DVE select chain, and tile 0
(packed with pure-modal-degree molecules) is a single range. Support DMAs
are split per consumer (bond rows ship as their own 64-descriptor tensor),
bond reduces + chunk2 assembly run on GpSimd, featT copies on Vector, so
ScalarE only triggers weight DMAs (no ACT_TABLE_LOAD stall).

See kernel_v2.py docstring for the core algorithm (active-atom compaction).
"""

import sys

sys.path.insert(0, "/opt/trn_rl_repo")

import numpy as np

B, A, D = 256, 128, 5
FA, FB, C = 256, 64, 256
F = FA + FB        # 320
FAUG = F + 1       # 321 (bias row)
NCORES = 8
BL = B // NCORES   # 32 molecules per core
TILES = 2          # compact tiles per core
CAP = 128          # compact slots per tile
BINW = FA + CAP    # per-bin blob width (256 atom cols + 128 gather cols)
NWARM = 22         # PE clock-gate warmup matmuls

_CACHE = {}


def _build_program(ranges, nbts):
    """ranges: per tile, tuple of (base, width, weight_block_index).
    nbts: gather bin count per tile."""
    from contextlib import ExitStack

    import concourse.bass as bass
    import concourse.tile as tile
    from concourse import bacc, mybir

    f32 = mybir.dt.float32
    OP = mybir.AluOpType
    bf16 = mybir.dt.bfloat16

    ND = max(di for tr in ranges for (_, _, di) in tr) + 1
    NW = ND * C
    tbase = [0]
    for t in range(TILES):
        tbase.append(tbase[-1] + nbts[t] * BINW)
    IBW = tbase[-1]          # total input blob width

    nc = bacc.Bacc("TRN2", target_bir_lowering=False, debug=False,
                   num_devices=NCORES)

    inblob_d = nc.dram_tensor("inblob", [128, IBW], bf16,
                              kind="ExternalInput")
    wblob_d = nc.dram_tensor("wblob", [128, 3 * NW], bf16,
                             kind="ExternalInput")
    bsum_d = nc.dram_tensor("bsum", [FB, TILES * CAP], bf16,
                            kind="ExternalInput")
    out_d = nc.dram_tensor("out", [CAP, TILES * C], f32,
                           kind="ExternalOutput")

    with tile.TileContext(nc) as tc, ExitStack() as ctx:
        consts = ctx.enter_context(tc.tile_pool(name="consts", bufs=1))
        pfeat = ctx.enter_context(tc.tile_pool(name="pfeat", bufs=2))
        ps_f = ctx.enter_context(
            tc.tile_pool(name="ps_f", bufs=2, space="PSUM"))
        ps_z = ctx.enter_context(
            tc.tile_pool(name="ps_z", bufs=2, space="PSUM"))

        # ---- PE warmup: dummy matmuls on a zeroed tile -------------------
        warm0 = consts.tile([128, 128], bf16)
        nc.vector.memset(warm0[:], 0.0)
        for i in range(NWARM):
            pw = ps_z.tile([128, 128], f32, tag="pw")
            nc.tensor.matmul(pw[:], warm0[:], warm0[:], start=True, stop=True)

        # ---- input DMAs --------------------------------------------------
        # tile-0 halves first on BOTH queues so its gather unblocks early
        inblob = consts.tile([128, IBW], bf16)
        for t in range(TILES):
            tw = nbts[t] * BINW
            hw = (nbts[t] // 2) * BINW if nbts[t] > 1 else tw
            pieces = ([(tbase[t], tbase[t] + hw),
                       (tbase[t] + hw, tbase[t] + tw)]
                      if hw < tw else [(tbase[t], tbase[t] + tw)])
            for pi, (lo, hi) in enumerate(pieces):
                eng = nc.sync if pi % 2 == 0 else nc.gpsimd
                eng.dma_start(out=inblob[:, lo:hi],
                              in_=inblob_d.ap()[:, lo:hi])
        bsum = consts.tile([FB, TILES * CAP], bf16)
        nc.scalar.dma_start(out=bsum[:], in_=bsum_d.ap()[:])
        wblob = consts.tile([128, 3 * NW], bf16)
        nc.scalar.dma_start(out=wblob[:], in_=wblob_d.ap()[:])
        w0 = wblob[:, 0:NW]
        w1 = wblob[:, NW:2 * NW]
        w2 = wblob[0:FAUG - 256, 2 * NW:3 * NW]

        out_sb = consts.tile([CAP, TILES * C], f32)
        nc.gpsimd.memset(out_sb[:], 0.0)

        # chunk2 = [bond sums; ones] [FB+1, CAP] per tile, built up front
        chunk2s = []
        for t in range(TILES):
            chunk2 = pfeat.tile([FB + 1, CAP], bf16, tag=f"c2_{t}")
            nc.gpsimd.tensor_copy(chunk2[0:FB, :],
                                  bsum[:, t * CAP:(t + 1) * CAP])
            nc.gpsimd.memset(chunk2[FB:FB + 1, :], 1.0)
            chunk2s.append(chunk2)

        # ---- per-tile pipeline -------------------------------------------
        for t in range(TILES):
            tb = tbase[t]
            # gather straight into featT layout: [feat chunk, slots]
            fps = []
            for k in range(2):
                fp = ps_f.tile([128, CAP], f32, tag=f"fp{k}")
                for bi in range(nbts[t]):
                    nc.tensor.matmul(
                        fp[:],
                        inblob[:, tb + bi * BINW + k * 128:
                               tb + bi * BINW + (k + 1) * 128],
                        inblob[:, tb + bi * BINW + FA:
                               tb + (bi + 1) * BINW],
                        start=(bi == 0), stop=(bi == nbts[t] - 1))
                fps.append(fp)
            featT = pfeat.tile([128, 2 * CAP], bf16, tag="ftT")
            with nc.allow_low_precision(reason="bf16 feat, tol 2e-2"):
                for k in range(2):
                    nc.vector.tensor_copy(featT[:, k * CAP:(k + 1) * CAP],
                                          fps[k][:])

            chunk2 = chunk2s[t]
            # dense: one 3-matmul chain per degree range of this tile
            pz = ps_z.tile([CAP, 512], f32, tag="pz")
            for (r0, rw, di) in ranges[t]:
                g0 = di * C
                nc.tensor.matmul(pz[r0:r0 + rw, 0:C],
                                 featT[:, r0:r0 + rw],
                                 w0[:, g0:g0 + C], start=True, stop=False)
                nc.tensor.matmul(pz[r0:r0 + rw, 0:C],
                                 featT[:, CAP + r0:CAP + r0 + rw],
                                 w1[:, g0:g0 + C], start=False, stop=False)
                nc.tensor.matmul(pz[r0:r0 + rw, 0:C],
                                 chunk2[:, r0:r0 + rw],
                                 w2[:, g0:g0 + C], start=False, stop=True)
            tot = ranges[t][-1][0] + ranges[t][-1][1]
            nc.vector.tensor_scalar(out_sb[0:tot, t * C:(t + 1) * C],
                                    pz[0:tot, 0:C], 0.0, None, OP.max)
            nc.scalar.dma_start(out=out_d.ap()[:, t * C:(t + 1) * C],
                                in_=out_sb[:, t * C:(t + 1) * C])

    nc.compile()
    return nc


def _get_nc(ranges, nbts):
    key = ("nc", tuple(tuple(tr) for tr in ranges), tuple(nbts))
    if key not in _CACHE:
        _CACHE[key] = _build_program(ranges, nbts)
    return _CACHE[key]


def _prep(atoms, bonds, edges, W, b):
    """Host-side: degree analysis, tile/bin packing, blob assembly."""
    import ml_dtypes

    bf16 = ml_dtypes.bfloat16
    atoms = np.asarray(atoms, dtype=np.float32)
    bonds = np.asarray(bonds, dtype=np.float32)
    edges = np.asarray(edges)
    W = np.asarray(W, dtype=np.float32)
    b = np.asarray(b, dtype=np.float32)

    deg = (edges != -1).sum(axis=-1)                      # (B, A)
    active = deg <= D - 1                                 # (B, A)
    deg_list = sorted(int(d) for d in np.unique(deg[active]))
    if not deg_list:
        deg_list = [D - 1]
    ND = len(deg_list)
    dpos = {d: i for i, d in enumerate(deg_list)}
    dcounts = {d: int((deg[active] == d).sum()) for d in deg_list}
    dmod = max(deg_list, key=lambda d: dcounts[d])

    waug = np.concatenate([W, b[:, None, :]], axis=1)     # (D, FAUG, C)
    waug = waug[deg_list].astype(bf16)                    # (ND, FAUG, C)
    NW = ND * C

    # ---- pass 1: per-core packing ------------------------------------
    cores = []
    need = np.zeros((NCORES, TILES, ND), dtype=int)  # slots per (t, deg)
    for c in range(NCORES):
        gm0 = c * BL
        acts, refs, refpos, pure = [], [], [], []
        for m in range(BL):
            gm = gm0 + m
            aidx = np.nonzero(active[gm])[0]
            acts.append(aidx)
            ra = set(aidx.tolist())
            for a in aidx:
                for e in edges[gm, a]:
                    if e >= 0:
                        ra.add(int(e))
            ra = sorted(ra)
            refs.append(ra)
            refpos.append({a: i for i, a in enumerate(ra)})
            pure.append(all(deg[gm, a] == dmod for a in aidx))

        slots = [0, 0]
        rsums = [0, 0]
        tmem = [[], []]

        def fits(t, m):
            return (slots[t] + len(acts[m]) <= CAP and
                    rsums[t] + len(refs[m]) <= 580)

        order = sorted(range(BL), key=lambda m: -len(refs[m]))
        for m in order:
            if len(acts[m]) == 0:
                continue
            if not pure[m]:
                if not fits(1, m):
                    raise RuntimeError(f"core {c}: tile packing failed")
                t = 1
            else:
                # fill tile 0 first so tile 1 keeps few slots per degree
                if fits(0, m):
                    t = 0
                elif fits(1, m):
                    t = 1
                else:
                    raise RuntimeError(f"core {c}: tile packing failed")
            tmem[t].append(m)
            slots[t] += len(acts[m])
            rsums[t] += len(refs[m])
            for a in acts[m]:
                need[c, t, dpos[int(deg[gm0 + m, a])]] += 1
        cores.append((acts, refs, refpos, tmem))

    # fixed per-(tile, degree) range capacities, shared across cores
    # PE matmul output BASE partitions may only be 0, 32 or 64. Place
    # small ranges first at 0/32, the big (modal) range last at the next
    # free base with room to grow to 128.
    caps = need.max(axis=0)                               # (TILES, ND)
    ranges = []
    for t in range(TILES):
        present = [di for di in range(ND) if caps[t, di] > 0]
        present.sort(key=lambda di: (int(caps[t, di]), deg_list[di]))
        assert len(present) <= 3, "too many degree ranges per tile"
        bases = [0, 32, 64][:len(present)]
        if present:
            bases[-1] = min(64, bases[-1])
        tr = []
        for i, di in enumerate(present):
            b0 = bases[i]
            lim = bases[i + 1] if i + 1 < len(present) else CAP
            w = int(caps[t, di])
            assert b0 + w <= lim, "range capacity overflow"
            tr.append((b0, w, di))
        # big range may span from its base to CAP
        ranges.append(tuple(tr))

    # bins per tile (first-fit decreasing); per-tile bin counts
    binned = []
    nbts = [1] * TILES
    for c in range(NCORES):
        acts, refs, refpos, tmem = cores[c]
        tb = []
        for t in range(TILES):
            bins = []
            for m in sorted(tmem[t], key=lambda m: -len(refs[m])):
                for bn in bins:
                    if bn[0] + len(refs[m]) <= 128:
                        bn[0] += len(refs[m])
                        bn[1].append(m)
                        break
                else:
                    bins.append([len(refs[m]), [m]])
            tb.append([bn[1] for bn in bins])
            nbts[t] = max(nbts[t], len(bins))
        binned.append(tb)
    tbase = [0]
    for t in range(TILES):
        tbase.append(tbase[-1] + nbts[t] * BINW)
    IBW = tbase[-1]

    rbase = {}  # (t, di) -> column base
    for t in range(TILES):
        for (b0, w, di) in ranges[t]:
            rbase[(t, di)] = b0

    in_maps = []
    scatter = []
    for c in range(NCORES):
        gm0 = c * BL
        acts, refs, refpos, tmem = cores[c]
        inblob = np.zeros((128, IBW), dtype=np.float32)
        bondsrc = np.zeros((FB, TILES * CAP), dtype=np.float32)
        sc_flat, sc_m, sc_a = [], [], []
        for t in range(TILES):
            cur = {di: 0 for di in range(ND)}
            for k, mols in enumerate(binned[c][t]):
                base = tbase[t] + k * BINW
                r = 0
                for m in mols:
                    gm = gm0 + m
                    ra = refs[m]
                    L = len(ra)
                    inblob[r:r + L, base:base + FA] = atoms[gm, ra]
                    for a in acts[m]:
                        di = dpos[int(deg[gm, a])]
                        col = rbase[(t, di)] + cur[di]
                        cur[di] += 1
                        inblob[r + refpos[m][a], base + FA + col] += 1.0
                        for dd in range(D):
                            e = edges[gm, a, dd]
                            if e >= 0:
                                inblob[r + refpos[m][int(e)],
                                       base + FA + col] += 1.0
                        bondsrc[:, t * CAP + col] = bonds[gm, a].sum(0)
                        sc_flat.append(t * CAP + col)
                        sc_m.append(gm)
                        sc_a.append(int(a))
                    r += L
        wblob = np.zeros((128, 3 * NW), dtype=np.float32)
        wblob[0:128, 0:NW] = \
            waug[:, 0:128, :].transpose(1, 0, 2).reshape(128, NW)
        wblob[0:128, NW:2 * NW] = \
            waug[:, 128:256, :].transpose(1, 0, 2).reshape(128, NW)
        wblob[0:FAUG - 256, 2 * NW:3 * NW] = \
            waug[:, 256:FAUG, :].transpose(1, 0, 2).reshape(FAUG - 256, NW)
        scatter.append((np.asarray(sc_flat), np.asarray(sc_m),
                        np.asarray(sc_a)))
        in_maps.append({
            "inblob": inblob.astype(bf16),
            "wblob": wblob.astype(bf16),
            "bsum": bondsrc.astype(bf16),
        })
    return ranges, nbts, in_maps, scatter


def run_sharded(atoms, bonds, edges, W, b, trace=False):
    """Run on the 8 NeuronCores; returns (output, BassKernelResults)."""
    from concourse.bass_utils import run_bass_kernel_spmd

    ranges, nbts, in_maps, scatter = _prep(atoms, bonds, edges, W, b)
    nc = _get_nc(ranges, nbts)
    res = run_bass_kernel_spmd(nc, in_maps, list(range(NCORES)), trace=trace)
    out = np.zeros((B, A, C), dtype=np.float32)
    for c in range(NCORES):
        sc_flat, sc_m, sc_a = scatter[c]
        oc = res.results[c]["out"]
        t_idx = sc_flat // CAP
        s_idx = sc_flat % CAP
        out[sc_m, sc_a] = oc[s_idx[:, None], (t_idx[:, None] * C +
                                              np.arange(C)[None, :])]
    return out, res


def kernel(atoms, bonds, edges, W, b):
    out, _ = run_sharded(atoms, bonds, edges, W, b)
    return out
